# revision 27
# baseline (speedup 1.0000x reference)
"""Trainium2 Bass kernel for nn_Attention_12146167513140.

Distributed dense attention over 8 NeuronCores.

Sharding: core c in 0..7 -> (b = c//4, head-pair hp = c%4).  Each core
computes the full [3072 q x 3072 k] attention for its 2 heads of its
batch, producing a partial output projection [3072, 256]; the host sums
the 4 partials per batch and adds b_out.

Device pipeline per core (all matmuls bf16, accumulation f32 in PSUM):
  A) kv = s2 @ Wkv_pair -> rms-norm k -> kT tiles (PE transpose),
     v (+ones col) -> vx tiles
  B) q = s1e @ Wq_pair -> rms-norm q -> qT tiles
  C) flash-style: scoresT[k,q] = kT.T @ qT (33rd contraction row carries
     the additive mask as a rank-1 term), exp on ScalarE (scale fused),
     PV: oT[h] += vx.T @ expT (ones column accumulates the softmax
     denominator Z), normalize by 1/Z
  D) out_partial = oT.T @ Wout_pair

Schedule: software-pipelined steady state emitting, per key-chunk step,
QK(kc+1) -> EXP(kc) -> one filler piece -> PV(kc-1), so the in-order PE
stream never sits behind the ScalarE exp.  All prologue/epilogue work
(projection+rms-norm chains, output projections) is chopped into filler
pieces with deadlines and drip-fed one per step into the attention
stream.  PSUM: 3 score buffers (6 banks), both heads' PV accumulators
packed into one bank via PE-array tile_position, 1 scratch bank.

Host-side prep: sinusoidal positional embedding (index arithmetic),
transposes, bf16 casts, mask row encoding.
"""

import contextlib
import ctypes
import sys
import types

import numpy as np
import ml_dtypes

import concourse.bacc as bacc
import concourse.mybir as mybir
from concourse import bass_utils
from concourse.tile import TileContext
from concourse.alu_op_type import AluOpType
from concourse.mybir import ActivationFunctionType as AF


def _ensure_trace_support():
    """The container's antenv package lacks axon_hooks; bass_utils
    imports it when tracing is requested (e.g. via BASS_TRACE).  Install
    a functional shim so a traced run works instead of crashing, and
    make the artifact upload a no-op (no bucket access here)."""
    try:
        import antenv.axon_hooks  # noqa: F401
        return
    except ImportError:
        pass
    mod = types.ModuleType("antenv.axon_hooks")
    mod._hook = None
    mod.set_axon_ntff_profile_hook = lambda h: setattr(mod, "_hook", h)
    mod.get_axon_ntff_profile_hook = lambda: mod._hook
    try:
        import antenv
        sys.modules["antenv.axon_hooks"] = mod
        antenv.axon_hooks = mod
    except ImportError:
        sys.modules["antenv.axon_hooks"] = mod

    def _ntff_hook(so_path):
        try:
            lib = ctypes.CDLL(so_path)
        except OSError:
            return None
        if not hasattr(lib, "axon_start_nrt_profile"):
            return None
        lib.axon_start_nrt_profile.argtypes = [ctypes.POINTER(ctypes.c_int64),
                                               ctypes.c_size_t]
        lib.axon_start_nrt_profile.restype = ctypes.c_int64
        lib.axon_stop_nrt_profile.argtypes = [ctypes.c_char_p]
        lib.axon_stop_nrt_profile.restype = ctypes.c_int64

        @contextlib.contextmanager
        def _hook(output_dir, device_ids):
            import jax
            jax.devices()
            if device_ids:
                ids = (ctypes.c_int64 * len(device_ids))(*device_ids)
                rc = lib.axon_start_nrt_profile(ids, len(device_ids))
            else:
                rc = lib.axon_start_nrt_profile(None, 0)
            if rc != 0:
                raise RuntimeError(f"axon_start_nrt_profile rc={rc}")
            try:
                yield
            finally:
                lib.axon_stop_nrt_profile(str(output_dir).encode())

        return _hook

    mod.set_axon_ntff_profile_hook(_ntff_hook("/opt/axon/libaxon_pjrt.so"))

    _orig_upload = bass_utils.upload_artifacts

    def _safe_upload(tmpdir):
        try:
            return _orig_upload(tmpdir)
        except Exception:
            return tmpdir

    bass_utils.upload_artifacts = _safe_upload


_ensure_trace_support()

AX = mybir.AxisListType
I32 = mybir.dt.int32
BF = mybir.dt.bfloat16
F32 = mybir.dt.float32
bf16 = ml_dtypes.bfloat16

B, N1, N2 = 2, 3072, 3072
C_S, H, D = 256, 8, 32
INF = 100000.0
EPS = 1e-8
SCALE = float(np.sqrt(1.0 / (3 * D)))

NCORES = 8
HPC = 2            # heads per core
KCH = N2 // 128    # 24 key chunks
QCH = N1 // 128    # 24 q row chunks
QB = 512           # q block for scores free dim
NQB = N1 // QB     # 6
VW = D + 1         # 33: v columns + ones column for Z
OT_OFF = (0, 64)   # oT partition offset per head (both in one PSUM bank)

_cache = {}


def _build(use_g2: bool):
    nc = bacc.Bacc("TRN2", target_bir_lowering=False, debug=False, num_devices=NCORES)

    s1T_d = nc.dram_tensor("s1T", [C_S, N1], BF, kind="ExternalInput")
    s2T_d = nc.dram_tensor("s2T", [C_S, N2], BF, kind="ExternalInput")
    wq_d = nc.dram_tensor("wq", [C_S, HPC * D], BF, kind="ExternalInput")
    wkv_d = nc.dram_tensor("wkv", [C_S, HPC * 2 * D], BF, kind="ExternalInput")
    wout_d = nc.dram_tensor("wout", [HPC * D, C_S], BF, kind="ExternalInput")
    qm_d = nc.dram_tensor("qm", [1, N1], BF, kind="ExternalInput")
    km_d = nc.dram_tensor("km", [1, N2], BF, kind="ExternalInput")
    id_d = nc.dram_tensor("ident", [128, 128], BF, kind="ExternalInput")
    if use_g2:
        g2_d = nc.dram_tensor("g2", [128, HPC * D], BF, kind="ExternalInput")
    zdr = [nc.dram_tensor(f"zscratch{h}", [1, N1], F32, kind="Internal")
           for h in range(HPC)]
    out_d = nc.dram_tensor("out", [N1, C_S], F32, kind="ExternalOutput")

    with TileContext(nc) as tc:
        with (
            tc.tile_pool(name="const", bufs=1) as cpool,
            tc.tile_pool(name="norm", bufs=6) as npool,
            tc.tile_pool(name="work", bufs=4) as work,
            tc.tile_pool(name="expp", bufs=4) as expp,
            tc.tile_pool(name="psA", bufs=2, space="PSUM") as psA,
            tc.tile_pool(name="psO", bufs=1, space="PSUM") as psO,
            tc.tile_pool(name="psC", bufs=3, space="PSUM") as psC,
        ):
            # ---- constants / staging ----
            ident = cpool.tile([128, 128], BF)
            nc.sync.dma_start(ident[:, :], id_d.ap())

            wq_sb = cpool.tile([128, HPC * D], BF, tag="wq")
            wq_sb2 = cpool.tile([128, HPC * D], BF, tag="wq2")
            nc.sync.dma_start(wq_sb[:, :], wq_d.ap()[0:128, :])
            nc.sync.dma_start(wq_sb2[:, :], wq_d.ap()[128:256, :])
            wkv_sb = cpool.tile([128, HPC * 2 * D], BF, tag="wkv")
            wkv_sb2 = cpool.tile([128, HPC * 2 * D], BF, tag="wkv2")
            nc.sync.dma_start(wkv_sb[:, :], wkv_d.ap()[0:128, :])
            nc.sync.dma_start(wkv_sb2[:, :], wkv_d.ap()[128:256, :])
            wout_sb = cpool.tile([HPC * D, C_S], BF, tag="wout")
            nc.sync.dma_start(wout_sb[:, :], wout_d.ap())
            if use_g2:
                g2_sb = cpool.tile([128, HPC * D], BF, tag="g2")
                nc.sync.dma_start(g2_sb[:, :], g2_d.ap())

            # s1/s2 staging: 4 wide slices per tensor-half, first slice of
            # every half dispatched first (the norm-chain prologue needs cols
            # 0-767 only), split across the Sync and ACT DGE sequencers.
            # These dispatches go out BEFORE the pad-row memsets below: a
            # memzero on the ACT queue would push the staging dispatches (and
            # hence every norm chain) out by its full duration.
            s1T = [cpool.tile([128, N1], BF, tag=f"s1T{i}", name=f"s1T{i}") for i in range(2)]
            s2T = [cpool.tile([128, N2], BF, tag=f"s2T{i}", name=f"s2T{i}") for i in range(2)]
            _dges = [nc.scalar, nc.sync]
            _dgei = 0
            for j in range(4):
                sl = slice(j * (N1 // 4), (j + 1) * (N1 // 4))
                for i in range(2):
                    _dges[_dgei % 2].dma_start(
                        s2T[i][:, sl], s2T_d.ap()[i * 128:(i + 1) * 128, sl])
                    _dgei += 1
                    _dges[_dgei % 2].dma_start(
                        s1T[i][:, sl], s1T_d.ap()[i * 128:(i + 1) * 128, sl])
                    _dgei += 1

            # per-head transposed tensors; row 32 carries the mask row.
            # Padded to 128 partitions (rows 33..127 zero) so the QK matmul
            # streams full-width K=128 (smaller K reconfigures the PE array
            # per matmul and measures ~1.6x slower).  The zero fill runs as
            # 32-row memsets on the otherwise-idle GpSimd/ACT engines — a
            # 2.4MB zeros DMA would double the prologue's DMA footprint.
            kT = [cpool.tile([128, N2], BF, tag=f"kT{h}", name=f"kT{h}") for h in range(HPC)]
            qT = [cpool.tile([128, N1], BF, tag=f"qT{h}", name=f"qT{h}") for h in range(HPC)]
            _zgrp = []
            for t in (qT[0], qT[1], kT[0], kT[1]):
                for r in (32, 64, 96):
                    _zgrp.append(t[r:r + 32, :])
            for i, ap in enumerate(_zgrp):
                if i in (0, 2, 4, 6, 8):
                    nc.gpsimd.memset(ap, 0.0)
                else:
                    nc.scalar.memzero(ap)
            for h in range(HPC):
                nc.sync.dma_start(kT[h][32:33, :], km_d.ap())
                nc.sync.dma_start(qT[h][32:33, :], qm_d.ap())

            # v-extended: per kchunk, per head: [v(32) | ones(1)] columns
            vx = cpool.tile([128, KCH * HPC * VW], BF, tag="vx")
            nc.gpsimd.memset(
                vx[:, :].rearrange("p (n w) -> p n w", w=VW)[:, :, 32:33], 1.0
            )

            oT_sb = cpool.tile([HPC * D, N1], BF, tag="oT")
            zrow = [cpool.tile([1, N1], F32, tag=f"zrow{h}", name=f"zrow{h}")
                    for h in range(HPC)]
            zq_bf = cpool.tile([33, QB], BF, tag="zq_bf")
            rzq = [cpool.tile([128, 33], F32, tag=f"rzq{i}", name=f"rzq{i}")
                   for i in range(4)]
            zp = [cpool.tile([128, QCH], F32, tag=f"zp{h}", name=f"zp{h}")
                  for h in range(HPC)]
            rzp = [cpool.tile([128, QCH], F32, tag=f"rzp{h}", name=f"rzp{h}")
                   for h in range(HPC)]

            # warm the Exp activation table while DMA streams in
            wt = npool.tile([1, 2], BF, tag="wt")
            nc.scalar.activation(wt[:, :], ident[0:1, 0:2], AF.Exp, scale=1.0)

            # ---- projection + rms-norm machinery ----
            NCHUNK = KCH + QCH  # 48
            kcp_all = cpool.tile([128, NCHUNK * HPC * D], F32, tag="kcp_all")
            ss_all = cpool.tile([128, NCHUNK * HPC], F32, tag="ss_all")
            sr_all = cpool.tile([128, NCHUNK * HPC], F32, tag="sr_all")
            rinv_all = cpool.tile([128, NCHUNK * HPC], F32, tag="rinv_all")

            def pass1(ci, kc, sT, w1, w2, vdst, act_copies=False):
                cp = nc.scalar.copy if act_copies else nc.vector.tensor_copy
                ncol = w1.shape[1]
                pp_t = psC.tile([128, QB], F32, tag="c", name=f"pp{ci}")
                pp = pp_t[:, 0:ncol]
                nc.tensor.matmul(pp, sT[0][:, kc * 128:(kc + 1) * 128], w1[:, :],
                                 start=True, stop=False)
                nc.tensor.matmul(pp, sT[1][:, kc * 128:(kc + 1) * 128], w2[:, :],
                                 start=False, stop=True)
                kcp = kcp_all[:, ci * HPC * D:(ci + 1) * HPC * D]
                cp(kcp.rearrange("p (h d) -> p h d", d=D),
                   pp.rearrange("p (h x) -> p h x", h=HPC)[:, :, 0:D])
                sq = npool.tile([128, HPC * D], F32, tag="sq", name=f"sq{ci}")
                nc.vector.tensor_tensor(sq[:, :], kcp, kcp, AluOpType.mult)
                nc.vector.reduce_sum(
                    ss_all[:, ci * HPC:(ci + 1) * HPC],
                    sq[:, :].rearrange("p (h d) -> p h d", d=D), axis=AX.X)
                if vdst is not None:  # kv: copy v columns into vx (+cast bf16)
                    cp(vdst[:, kc * HPC * VW:(kc + 1) * HPC * VW]
                       .rearrange("p (h w) -> p h w", w=VW)[:, :, 0:D],
                       pp.rearrange("p (h x) -> p h x", h=HPC)[:, :, D:2 * D])

            def rsqrt_batch(sl, bid):
                # rinv = 1/sqrt(ss/D + eps) entirely on DVE (bit-trick seed +
                # 2 Newton steps, ~5e-6 rel err).  Keeps Sqrt off ScalarE so
                # the in-order ACT stream carries nothing but Exp ops.
                w = sl.stop - sl.start
                x = sr_all[:, sl]
                nc.vector.tensor_scalar(x, ss_all[:, sl], 1.0 / D, EPS,
                                        AluOpType.mult, AluOpType.add)
                t = npool.tile([128, NCHUNK * HPC], I32, tag="nrt",
                               name=f"nrt{bid}")
                nc.vector.tensor_scalar(t[:, 0:w], x.bitcast(I32), 1, None,
                                        AluOpType.arith_shift_right)
                u = npool.tile([128, NCHUNK * HPC], I32, tag="nru",
                               name=f"nru{bid}")
                nc.vector.tensor_scalar(u[:, 0:w], t[:, 0:w], -1, 0x5F3759DF,
                                        AluOpType.mult, AluOpType.add)
                y = u[:, 0:w].bitcast(F32)
                for it in range(2):
                    a = npool.tile([128, NCHUNK * HPC], F32, tag="nra",
                                   name=f"nra{bid}_{it}")
                    nc.vector.tensor_tensor(a[:, 0:w], y, y, AluOpType.mult)
                    b = npool.tile([128, NCHUNK * HPC], F32, tag="nrb",
                                   name=f"nrb{bid}_{it}")
                    nc.vector.tensor_tensor(b[:, 0:w], a[:, 0:w], x,
                                            AluOpType.mult)
                    c = npool.tile([128, NCHUNK * HPC], F32, tag="nrc",
                                   name=f"nrc{bid}_{it}")
                    nc.vector.tensor_scalar(c[:, 0:w], b[:, 0:w], -0.5, 1.5,
                                            AluOpType.mult, AluOpType.add)
                    dst = (npool.tile([128, NCHUNK * HPC], F32, tag="nry",
                                      name=f"nry{bid}_{it}")
                           if it == 0 else None)
                    out = dst[:, 0:w] if it == 0 else rinv_all[:, sl]
                    nc.vector.tensor_tensor(out, y, c[:, 0:w], AluOpType.mult)
                    y = out

            def pass2_pair(kc0, kc1, kvside, act_copies=False):
                # normalize two chunks and transpose them with ONE PE
                # transpose: pre2 cols [h0@kc0 | h1@kc0 | h0@kc1 | h1@kc1]
                dstT = kT if kvside else qT
                pre2 = npool.tile([128, 128], BF, tag="pre2",
                                  name=f"pre2{'kv' if kvside else 'q'}{kc0}")
                for j, kc in enumerate((kc0, kc1)):
                    ci = kc if kvside else KCH + kc
                    kcp = kcp_all[:, ci * HPC * D:(ci + 1) * HPC * D]
                    for h in range(HPC):
                        nc.vector.tensor_scalar(
                            pre2[:, j * 64 + h * D:j * 64 + (h + 1) * D],
                            kcp[:, h * D:(h + 1) * D],
                            rinv_all[:, ci * HPC + h:ci * HPC + h + 1], None,
                            AluOpType.mult)
                    if use_g2 and not kvside:  # q side carries the gq*gk factor
                        nc.vector.tensor_tensor(
                            pre2[:, j * 64:(j + 1) * 64],
                            pre2[:, j * 64:(j + 1) * 64], g2_sb[:, :],
                            AluOpType.mult)
                tp_t = psC.tile([128, 128], BF, tag="c",
                                name=f"tp{'kv' if kvside else 'q'}{kc0}")
                nc.tensor.transpose(tp_t[:, :], pre2[:, :], ident[:, :])
                for j, kc in enumerate((kc0, kc1)):
                    for h in range(HPC):
                        if act_copies:
                            nc.scalar.copy(
                                dstT[h][0:D, kc * 128:(kc + 1) * 128],
                                tp_t[j * 64 + h * D:j * 64 + (h + 1) * D, :])
                        else:
                            nc.vector.tensor_copy(
                                dstT[h][0:D, kc * 128:(kc + 1) * 128],
                                tp_t[j * 64 + h * D:j * 64 + (h + 1) * D, :])

            def norm_group(chunks, kvside):
                # emit a full chain inline (prologue use)
                for kc in chunks:
                    if kvside:
                        pass1(kc, kc, s2T, wkv_sb, wkv_sb2, vx)
                    else:
                        pass1(KCH + kc, kc, s1T, wq_sb, wq_sb2, None)
                ci0 = (chunks[0] if kvside else KCH + chunks[0]) * HPC
                ci1 = (chunks[-1] if kvside else KCH + chunks[-1]) * HPC + HPC
                rsqrt_batch(slice(ci0, ci1), f"{'kv' if kvside else 'q'}{ci0}")
                for i in range(0, len(chunks), 2):
                    pass2_pair(chunks[i], chunks[i + 1], kvside)

            def group_pieces(chunks, kvside, deadline=None, act_copies=False):
                # same chain as norm_group, chopped into filler pieces
                ps = []
                for kc in chunks:
                    if kvside:
                        ps.append((lambda kc=kc: pass1(kc, kc, s2T, wkv_sb, wkv_sb2, vx,
                                                       act_copies=act_copies),
                                   deadline))
                    else:
                        ps.append((lambda kc=kc: pass1(KCH + kc, kc, s1T, wq_sb, wq_sb2, None,
                                                       act_copies=act_copies),
                                   deadline))
                ci0 = (chunks[0] if kvside else KCH + chunks[0]) * HPC
                ci1 = (chunks[-1] if kvside else KCH + chunks[-1]) * HPC + HPC
                ps.append((lambda: rsqrt_batch(slice(ci0, ci1),
                                               f"{'kv' if kvside else 'q'}{ci0}"),
                           deadline))
                for i in range(0, len(chunks), 2):
                    ps.append((lambda i=i: pass2_pair(chunks[i], chunks[i + 1], kvside,
                                                      act_copies=act_copies),
                               deadline))
                return ps

            def proj_out(qc, dge=None):
                # out rows qc*128..: both heads' projection, normalized by 1/Z
                osl = slice(qc * 128, (qc + 1) * 128)
                if qc >= (NQB - 1) * 4:
                    rz0 = rzq[qc % 4][:, 0:1]
                    rz1 = rzq[qc % 4][:, 32:33]
                else:
                    rz0 = rzp[0][:, qc:qc + 1]
                    rz1 = rzp[1][:, qc:qc + 1]
                op0 = psC.tile([128, C_S], F32, tag="c", name=f"op0_{qc}")
                nc.tensor.matmul(op0[:, :], oT_sb[0:D, osl], wout_sb[0:D, :],
                                 start=True, stop=True)
                op1 = psC.tile([128, C_S], F32, tag="c", name=f"op1_{qc}")
                nc.tensor.matmul(op1[:, :], oT_sb[D:2 * D, osl],
                                 wout_sb[D:2 * D, :], start=True, stop=True)
                t0 = work.tile([128, C_S], F32, tag="t0", name=f"t0_{qc}")
                nc.vector.tensor_scalar(t0[:, :], op0[:, :], rz0, None,
                                        AluOpType.mult)
                ops = work.tile([128, C_S], F32, tag="osb", name=f"osb_{qc}")
                nc.vector.scalar_tensor_tensor(
                    ops[:, :], op1[:, :], rz1, t0[:, :],
                    AluOpType.mult, AluOpType.add)
                if dge is not None:
                    nparts = 4 if qc == NQB * 4 - 1 else 2
                    step = 128 // nparts
                    for pi in range(nparts):
                        eng = nc.sync if pi % 2 == 0 else nc.scalar
                        r0 = qc * 128 + pi * step
                        eng.dma_start(out_d.ap()[r0:r0 + step, :],
                                      ops[pi * step:(pi + 1) * step, :])
                else:
                    nc.sync.dma_start(out_d.ap()[osl, :], ops[:, :])

            # ---- attention pipeline ----
            def attend(qb, fillers):
                fillers = list(fillers)
                nf = len(fillers)
                done = 0
                qsl = slice(qb * QB, (qb + 1) * QB)
                oTt = psO.tile([128, QB], F32, tag="oT", name=f"oT{qb}")
                oT = [oTt[OT_OFF[h]:OT_OFF[h] + VW, :] for h in range(HPC)]
                scs = {}
                exs = {}

                def qk(kc):
                    sc = psA.tile([128, HPC * QB], F32, tag="sc",
                                  name=f"sc{qb}_{kc}")
                    for h in range(HPC):
                        nc.tensor.matmul(
                            sc[:, h * QB:(h + 1) * QB],
                            kT[h][:, kc * 128:(kc + 1) * 128],
                            qT[h][:, qsl],
                            start=True, stop=True)
                    scs[kc] = sc

                def pv(kc):
                    ex = exs.pop(kc)
                    for h in range(HPC):
                        nc.tensor.matmul(
                            oT[h][:, :],
                            vx[:, (kc * HPC + h) * VW:(kc * HPC + h + 1) * VW],
                            ex[:, h * QB:(h + 1) * QB],
                            start=(kc == 0), stop=(kc == KCH - 1),
                            skip_group_check=True)

                qk(0)
                for kc in range(KCH):
                    if kc + 1 < KCH:
                        qk(kc + 1)
                    sc = scs.pop(kc)
                    ex = expp.tile([128, HPC * QB], BF, tag="ex",
                                   name=f"ex{qb}_{kc}")
                    nc.scalar.activation(ex[:, :], sc[:, :], AF.Exp, scale=SCALE)
                    exs[kc] = ex
                    while done < nf and (
                            (fillers[done][1] is not None and fillers[done][1] <= kc)
                            or done * KCH < (kc + 1) * nf):
                        fillers[done][0]()
                        done += 1
                    if kc >= 1:
                        pv(kc - 1)
                pv(KCH - 1)
                while done < nf:
                    fillers[done][0]()
                    done += 1

                # epilogue: unnormalized o -> sbuf bf16; Z row -> zrow;
                # 1/Z into q-partition-major layout via a DRAM bounce.  The
                # final q-block instead transposes Z on the (by-then idle) PE
                # so the tail doesn't wait on two serialized DMA round trips.
                if qb == NQB - 1:
                    # tail: fan the epilogue copies across DVE/GpSimd/ACT so
                    # they don't serialize on one engine, and skip zrow (the
                    # DRAM bounce isn't used for the final q-block).
                    nc.scalar.copy(oT_sb[0 * D:1 * D, qsl], oT[0][0:D, :])
                    nc.scalar.copy(oT_sb[1 * D:2 * D, qsl], oT[1][0:D, :])
                    nc.vector.tensor_copy(zq_bf[0:1, :], oT[0][D:VW, :])
                    nc.vector.tensor_copy(zq_bf[32:33, :], oT[1][D:VW, :])
                    for qc4 in range(4):
                        tpz = psC.tile([128, 64], BF, tag="c", name=f"tpz{qc4}")
                        nc.tensor.transpose(
                            tpz[:, 0:33],
                            zq_bf[0:33, qc4 * 128:(qc4 + 1) * 128],
                            ident[0:33, 0:33])
                        nc.vector.reciprocal(
                            rzq[qc4][:, :], tpz[:, 0:33])
                else:
                    for h in range(HPC):
                        nc.vector.tensor_copy(oT_sb[h * D:(h + 1) * D, qsl],
                                              oT[h][0:D, :])
                        nc.vector.tensor_copy(zrow[h][0:1, qsl],
                                              oT[h][D:VW, :])
                    for h in range(HPC):
                        nc.sync.dma_start(zdr[h].ap()[0:1, qsl], zrow[h][0:1, qsl])
                        nc.sync.dma_start(
                            zp[h][:, qb * 4:(qb + 1) * 4],
                            zdr[h].ap()[0:1, qsl].rearrange("o (c p) -> o p c", p=128)[0])
                        nc.vector.reciprocal(rzp[h][:, qb * 4:(qb + 1) * 4],
                                             zp[h][:, qb * 4:(qb + 1) * 4])

            # ---- schedule ----
            # prologue: kv chunks 0-3 + q chunks 0-3 fully normalized.
            # All pass1 chains go first (they only gate on staging slices);
            # the rsqrt batches land in the window where the pad-row memsets
            # still hog the engines, so nothing downstream waits on them.
            for kc in range(4):
                pass1(kc, kc, s2T, wkv_sb, wkv_sb2, vx)
                pass1(KCH + kc, kc, s1T, wq_sb, wq_sb2, None)
            rsqrt_batch(slice(0, 8), "kv0")
            rsqrt_batch(slice(KCH * HPC, KCH * HPC + 8), "q0")
            pass2_pair(0, 1, False, act_copies=True)
            pass2_pair(2, 3, False, act_copies=True)
            pass2_pair(0, 1, True, act_copies=True)

            # attend(0): kv chunks 4-23 JIT (chunk c's pieces forced by step
            # c-2 so the QK that reads kT[c] sees the writes), then q 4-7.
            f0 = [(lambda: pass2_pair(2, 3, True, act_copies=True), 0)]
            for g0 in range(4, 24, 4):
                g = list(range(g0, g0 + 4))
                f0 += group_pieces(g, kvside=True, deadline=max(0, g[0] - 2),
                                   act_copies=True)
            f0 += group_pieces([4, 5, 6, 7], kvside=False, act_copies=True)
            attend(0, f0)

            # attend(1..4): one q-chunk group each; attend(2+) also carry the
            # out-projections of qb-2 (their 1/Z landed ~a full attend ago).
            for qb in range(1, NQB):
                fq = []
                g0 = 4 + 4 * qb
                if g0 < QCH:
                    fq += group_pieces(list(range(g0, g0 + 4)), kvside=False)
                if qb >= 2:
                    for qc in range((qb - 2) * 4, (qb - 1) * 4):
                        fq.append((lambda qc=qc: proj_out(qc), None))
                if qb == NQB - 1:
                    for qc in range((qb - 1) * 4, qb * 4):
                        fq.append((lambda qc=qc: proj_out(qc), None))
                attend(qb, fq)
            for qc in range((NQB - 1) * 4, NQB * 4):
                proj_out(qc, dge=True)

    nc.compile()
    return nc


def _host_prep(inputs):
    s1 = np.asarray(inputs["s1"], np.float32)
    s2 = np.asarray(inputs["s2"], np.float32)
    ridx1 = np.asarray(inputs["ridx1"], np.int32)
    ct1 = np.asarray(inputs["ct1"], np.int32)
    mask1 = np.asarray(inputs["mask1"], np.int32)
    mask2 = np.asarray(inputs["mask2"], np.int32)
    Wq = np.asarray(inputs["Wq"], np.float32)
    Wkv = np.asarray(inputs["Wkv"], np.float32)
    Wout = np.asarray(inputs["Wout"], np.float32)
    gq = np.asarray(inputs["gq"], np.float32)
    gk = np.asarray(inputs["gk"], np.float32)

    ct_idx = np.take_along_axis(ridx1, ct1[:, None], axis=1)
    pos = (ridx1 - ct_idx).astype(np.float32)
    half = C_S // 2
    freqs = np.exp(-np.log(10000.0) * np.arange(half, dtype=np.float32) / half)
    ang = pos[..., None] * freqs
    s1e = s1 + np.concatenate([np.sin(ang), np.cos(ang)], axis=-1).astype(np.float32)

    m1 = mask1.astype(np.float32)
    km = (mask2.astype(np.float32) - 1.0) * INF / SCALE

    g2 = gq * gk
    use_g2 = not np.allclose(g2, 1.0)

    ident = np.eye(128, dtype=bf16)
    in_maps = []
    for c in range(NCORES):
        b, hp = c // 4, c % 4
        m = {
            "s1T": np.ascontiguousarray(s1e[b].T).astype(bf16),
            "s2T": np.ascontiguousarray(s2[b].T).astype(bf16),
            "wq": np.ascontiguousarray(Wq[:, hp * HPC * D:(hp + 1) * HPC * D]).astype(bf16),
            "wkv": np.ascontiguousarray(Wkv[:, hp * HPC * 2 * D:(hp + 1) * HPC * 2 * D]).astype(bf16),
            "wout": np.ascontiguousarray(Wout[hp * HPC * D:(hp + 1) * HPC * D, :]).astype(bf16),
            "qm": m1[b][None, :].astype(bf16),
            "km": km[b][None, :].astype(bf16),
            "ident": ident,
        }
        if use_g2:
            m["g2"] = np.tile(g2[None, hp * HPC * D:(hp + 1) * HPC * D], (128, 1)).astype(bf16)
        in_maps.append(m)
    return in_maps, use_g2, np.asarray(inputs["b_out"], np.float32)


def _run(inputs, trace=False, **kw):
    in_maps, use_g2, b_out = _host_prep(inputs)
    key = ("nc", use_g2)
    if key not in _cache:
        _cache[key] = _build(use_g2)
    nc = _cache[key]
    res = bass_utils.run_bass_kernel_spmd(
        nc, in_maps, core_ids=list(range(NCORES)), trace=trace, **kw)
    out = np.zeros((B, N1, C_S), np.float32)
    for c in range(NCORES):
        out[c // 4] += res.results[c]["out"]
    out += b_out[None, None, :]
    return out, res


def kernel(**inputs) -> np.ndarray:
    out, _ = _run(inputs, trace=False)
    return out


# revision 28
# speedup vs baseline: 1.0801x; 1.0801x over previous
"""Trainium2 Bass kernel for nn_Attention_12146167513140.

Distributed dense attention over 8 NeuronCores.

Sharding: core c in 0..7 -> (b = c//4, head-pair hp = c%4).  Each core
computes the full [3072 q x 3072 k] attention for its 2 heads of its
batch, producing a partial output projection [3072, 256]; the host sums
the 4 partials per batch and adds b_out.

Device pipeline per core (all matmuls bf16, accumulation f32 in PSUM):
  A) kv = s2 @ Wkv_pair -> rms-norm k -> kT tiles (PE transpose),
     v (+ones col) -> vx tiles
  B) q = s1e @ Wq_pair -> rms-norm q -> qT tiles
  C) flash-style: scoresT[k,q] = kT.T @ qT (33rd contraction row carries
     the additive mask as a rank-1 term), exp on ScalarE (scale fused),
     PV: oT[h] += vx.T @ expT (ones column accumulates the softmax
     denominator Z), normalize by 1/Z
  D) out_partial = oT.T @ Wout_pair

Schedule: software-pipelined steady state emitting, per key-chunk step,
QK(kc+1) -> EXP(kc) -> one filler piece -> PV(kc-1), so the in-order PE
stream never sits behind the ScalarE exp.  All prologue/epilogue work
(projection+rms-norm chains, output projections) is chopped into filler
pieces with deadlines and drip-fed one per step into the attention
stream.  PSUM: 3 score buffers (6 banks), both heads' PV accumulators
packed into one bank via PE-array tile_position, 1 scratch bank.

Host-side prep: sinusoidal positional embedding (index arithmetic),
transposes, bf16 casts, mask row encoding.
"""

import contextlib
import ctypes
import sys
import types

import numpy as np
import ml_dtypes

import concourse.bacc as bacc
import concourse.mybir as mybir
from concourse import bass_utils
from concourse.tile import TileContext
from concourse.alu_op_type import AluOpType
from concourse.mybir import ActivationFunctionType as AF


def _ensure_trace_support():
    """The container's antenv package lacks axon_hooks; bass_utils
    imports it when tracing is requested (e.g. via BASS_TRACE).  Install
    a functional shim so a traced run works instead of crashing, and
    make the artifact upload a no-op (no bucket access here)."""
    try:
        import antenv.axon_hooks  # noqa: F401
        return
    except ImportError:
        pass
    mod = types.ModuleType("antenv.axon_hooks")
    mod._hook = None
    mod.set_axon_ntff_profile_hook = lambda h: setattr(mod, "_hook", h)
    mod.get_axon_ntff_profile_hook = lambda: mod._hook
    try:
        import antenv
        sys.modules["antenv.axon_hooks"] = mod
        antenv.axon_hooks = mod
    except ImportError:
        sys.modules["antenv.axon_hooks"] = mod

    def _ntff_hook(so_path):
        try:
            lib = ctypes.CDLL(so_path)
        except OSError:
            return None
        if not hasattr(lib, "axon_start_nrt_profile"):
            return None
        lib.axon_start_nrt_profile.argtypes = [ctypes.POINTER(ctypes.c_int64),
                                               ctypes.c_size_t]
        lib.axon_start_nrt_profile.restype = ctypes.c_int64
        lib.axon_stop_nrt_profile.argtypes = [ctypes.c_char_p]
        lib.axon_stop_nrt_profile.restype = ctypes.c_int64

        @contextlib.contextmanager
        def _hook(output_dir, device_ids):
            import jax
            jax.devices()
            if device_ids:
                ids = (ctypes.c_int64 * len(device_ids))(*device_ids)
                rc = lib.axon_start_nrt_profile(ids, len(device_ids))
            else:
                rc = lib.axon_start_nrt_profile(None, 0)
            if rc != 0:
                raise RuntimeError(f"axon_start_nrt_profile rc={rc}")
            try:
                yield
            finally:
                lib.axon_stop_nrt_profile(str(output_dir).encode())

        return _hook

    mod.set_axon_ntff_profile_hook(_ntff_hook("/opt/axon/libaxon_pjrt.so"))

    _orig_upload = bass_utils.upload_artifacts

    def _safe_upload(tmpdir):
        try:
            return _orig_upload(tmpdir)
        except Exception:
            return tmpdir

    bass_utils.upload_artifacts = _safe_upload


_ensure_trace_support()

AX = mybir.AxisListType
I32 = mybir.dt.int32
BF = mybir.dt.bfloat16
F32 = mybir.dt.float32
bf16 = ml_dtypes.bfloat16

B, N1, N2 = 2, 3072, 3072
C_S, H, D = 256, 8, 32
INF = 100000.0
EPS = 1e-8
SCALE = float(np.sqrt(1.0 / (3 * D)))

NCORES = 8
HPC = 2            # heads per core
KCH = N2 // 128    # 24 key chunks
QCH = N1 // 128    # 24 q row chunks
QB = 512           # q block for scores free dim
NQB = N1 // QB     # 6
VW = D + 1         # 33: v columns + ones column for Z
OT_OFF = (0, 64)   # oT partition offset per head (both in one PSUM bank)

_cache = {}


def _build(use_g2: bool):
    nc = bacc.Bacc("TRN2", target_bir_lowering=False, debug=False, num_devices=NCORES)

    s1T_d = nc.dram_tensor("s1T", [C_S, N1], BF, kind="ExternalInput")
    s2T_d = nc.dram_tensor("s2T", [C_S, N2], BF, kind="ExternalInput")
    wq_d = nc.dram_tensor("wq", [C_S, HPC * D], BF, kind="ExternalInput")
    wkv_d = nc.dram_tensor("wkv", [C_S, HPC * 2 * D], BF, kind="ExternalInput")
    wout_d = nc.dram_tensor("wout", [HPC * D, C_S], BF, kind="ExternalInput")
    qm_d = nc.dram_tensor("qm", [1, N1], BF, kind="ExternalInput")
    km_d = nc.dram_tensor("km", [1, N2], BF, kind="ExternalInput")
    id_d = nc.dram_tensor("ident", [128, 128], BF, kind="ExternalInput")
    if use_g2:
        g2_d = nc.dram_tensor("g2", [128, HPC * D], BF, kind="ExternalInput")
    zdr = [nc.dram_tensor(f"zscratch{h}", [1, N1], F32, kind="Internal")
           for h in range(HPC)]
    out_d = nc.dram_tensor("out", [N1, C_S], F32, kind="ExternalOutput")

    with TileContext(nc) as tc:
        with (
            tc.tile_pool(name="const", bufs=1) as cpool,
            tc.tile_pool(name="norm", bufs=6) as npool,
            tc.tile_pool(name="work", bufs=4) as work,
            tc.tile_pool(name="expp", bufs=4) as expp,
            tc.tile_pool(name="psA", bufs=2, space="PSUM") as psA,
            tc.tile_pool(name="psO", bufs=1, space="PSUM") as psO,
            tc.tile_pool(name="psC", bufs=3, space="PSUM") as psC,
        ):
            # ---- constants / staging ----
            ident = cpool.tile([128, 128], BF)
            nc.sync.dma_start(ident[:, :], id_d.ap())

            wq_sb = cpool.tile([128, HPC * D], BF, tag="wq")
            wq_sb2 = cpool.tile([128, HPC * D], BF, tag="wq2")
            nc.sync.dma_start(wq_sb[:, :], wq_d.ap()[0:128, :])
            nc.sync.dma_start(wq_sb2[:, :], wq_d.ap()[128:256, :])
            wkv_sb = cpool.tile([128, HPC * 2 * D], BF, tag="wkv")
            wkv_sb2 = cpool.tile([128, HPC * 2 * D], BF, tag="wkv2")
            nc.sync.dma_start(wkv_sb[:, :], wkv_d.ap()[0:128, :])
            nc.sync.dma_start(wkv_sb2[:, :], wkv_d.ap()[128:256, :])
            wout_sb = cpool.tile([HPC * D, C_S], BF, tag="wout")
            nc.sync.dma_start(wout_sb[:, :], wout_d.ap())
            if use_g2:
                g2_sb = cpool.tile([128, HPC * D], BF, tag="g2")
                nc.sync.dma_start(g2_sb[:, :], g2_d.ap())

            # s1/s2 staging: 4 wide slices per tensor-half, first slice of
            # every half dispatched first (the norm-chain prologue needs cols
            # 0-767 only), split across the Sync and ACT DGE sequencers.
            # These dispatches go out BEFORE the pad-row memsets below: a
            # memzero on the ACT queue would push the staging dispatches (and
            # hence every norm chain) out by its full duration.
            s1T = [cpool.tile([128, N1], BF, tag=f"s1T{i}", name=f"s1T{i}") for i in range(2)]
            s2T = [cpool.tile([128, N2], BF, tag=f"s2T{i}", name=f"s2T{i}") for i in range(2)]
            _dges = [nc.scalar, nc.sync]
            _dgei = 0
            for j in range(4):
                sl = slice(j * (N1 // 4), (j + 1) * (N1 // 4))
                for i in range(2):
                    _dges[_dgei % 2].dma_start(
                        s2T[i][:, sl], s2T_d.ap()[i * 128:(i + 1) * 128, sl])
                    _dgei += 1
                    _dges[_dgei % 2].dma_start(
                        s1T[i][:, sl], s1T_d.ap()[i * 128:(i + 1) * 128, sl])
                    _dgei += 1

            # per-head transposed tensors; row 32 carries the mask row.
            # Padded to 128 partitions (rows 33..127 zero) so the QK matmul
            # streams full-width K=128 (smaller K reconfigures the PE array
            # per matmul and measures ~1.6x slower).  The zero fill runs as
            # 32-row memsets on the otherwise-idle GpSimd/ACT engines — a
            # 2.4MB zeros DMA would double the prologue's DMA footprint.
            kT = [cpool.tile([128, N2], BF, tag=f"kT{h}", name=f"kT{h}") for h in range(HPC)]
            qT = [cpool.tile([128, N1], BF, tag=f"qT{h}", name=f"qT{h}") for h in range(HPC)]
            _zgrp = []
            for t in (qT[0], qT[1], kT[0], kT[1]):
                for r in (32, 64, 96):
                    _zgrp.append(t[r:r + 32, :])
            for i, ap in enumerate(_zgrp):
                if i in (0, 2, 4, 6, 8):
                    nc.gpsimd.memset(ap, 0.0)
                else:
                    nc.scalar.memzero(ap)
            for h in range(HPC):
                nc.sync.dma_start(kT[h][32:33, :], km_d.ap())
                nc.sync.dma_start(qT[h][32:33, :], qm_d.ap())

            # v-extended: per kchunk, per head: [v(32) | ones(1)] columns
            vx = cpool.tile([128, KCH * HPC * VW], BF, tag="vx")
            nc.gpsimd.memset(
                vx[:, :].rearrange("p (n w) -> p n w", w=VW)[:, :, 32:33], 1.0
            )

            oT_sb = cpool.tile([HPC * D, N1], BF, tag="oT")
            zrow = [cpool.tile([1, N1], F32, tag=f"zrow{h}", name=f"zrow{h}")
                    for h in range(HPC)]
            zq_bf = cpool.tile([33, QB], BF, tag="zq_bf")
            rzq = [cpool.tile([128, 33], F32, tag=f"rzq{i}", name=f"rzq{i}")
                   for i in range(4)]
            zp = [cpool.tile([128, QCH], F32, tag=f"zp{h}", name=f"zp{h}")
                  for h in range(HPC)]
            rzp = [cpool.tile([128, QCH], F32, tag=f"rzp{h}", name=f"rzp{h}")
                   for h in range(HPC)]

            # warm the Exp activation table while DMA streams in
            wt = npool.tile([1, 2], BF, tag="wt")
            nc.scalar.activation(wt[:, :], ident[0:1, 0:2], AF.Exp, scale=1.0)

            # ---- projection + rms-norm machinery ----
            NCHUNK = KCH + QCH  # 48
            kcp_all = cpool.tile([128, NCHUNK * HPC * D], F32, tag="kcp_all")
            ss_all = cpool.tile([128, NCHUNK * HPC], F32, tag="ss_all")
            sr_all = cpool.tile([128, NCHUNK * HPC], F32, tag="sr_all")
            rinv_all = cpool.tile([128, NCHUNK * HPC], F32, tag="rinv_all")

            def pass1(ci, kc, sT, w1, w2, vdst, act_copies=False):
                cp = nc.scalar.copy if act_copies else nc.vector.tensor_copy
                ncol = w1.shape[1]
                pp_t = psC.tile([128, QB], F32, tag="c", name=f"pp{ci}")
                pp = pp_t[:, 0:ncol]
                nc.tensor.matmul(pp, sT[0][:, kc * 128:(kc + 1) * 128], w1[:, :],
                                 start=True, stop=False)
                nc.tensor.matmul(pp, sT[1][:, kc * 128:(kc + 1) * 128], w2[:, :],
                                 start=False, stop=True)
                kcp = kcp_all[:, ci * HPC * D:(ci + 1) * HPC * D]
                cp(kcp.rearrange("p (h d) -> p h d", d=D),
                   pp.rearrange("p (h x) -> p h x", h=HPC)[:, :, 0:D])
                sq = npool.tile([128, HPC * D], F32, tag="sq", name=f"sq{ci}")
                nc.vector.tensor_tensor(sq[:, :], kcp, kcp, AluOpType.mult)
                nc.vector.reduce_sum(
                    ss_all[:, ci * HPC:(ci + 1) * HPC],
                    sq[:, :].rearrange("p (h d) -> p h d", d=D), axis=AX.X)
                if vdst is not None:  # kv: copy v columns into vx (+cast bf16)
                    cp(vdst[:, kc * HPC * VW:(kc + 1) * HPC * VW]
                       .rearrange("p (h w) -> p h w", w=VW)[:, :, 0:D],
                       pp.rearrange("p (h x) -> p h x", h=HPC)[:, :, D:2 * D])

            def rsqrt_batch(sl, bid):
                # rinv = 1/sqrt(ss/D + eps) entirely on DVE (bit-trick seed +
                # 2 Newton steps, ~5e-6 rel err).  Keeps Sqrt off ScalarE so
                # the in-order ACT stream carries nothing but Exp ops.
                w = sl.stop - sl.start
                x = sr_all[:, sl]
                nc.vector.tensor_scalar(x, ss_all[:, sl], 1.0 / D, EPS,
                                        AluOpType.mult, AluOpType.add)
                t = npool.tile([128, NCHUNK * HPC], I32, tag="nrt",
                               name=f"nrt{bid}")
                nc.vector.tensor_scalar(t[:, 0:w], x.bitcast(I32), 1, None,
                                        AluOpType.arith_shift_right)
                u = npool.tile([128, NCHUNK * HPC], I32, tag="nru",
                               name=f"nru{bid}")
                nc.vector.tensor_scalar(u[:, 0:w], t[:, 0:w], -1, 0x5F3759DF,
                                        AluOpType.mult, AluOpType.add)
                y = u[:, 0:w].bitcast(F32)
                for it in range(2):
                    a = npool.tile([128, NCHUNK * HPC], F32, tag="nra",
                                   name=f"nra{bid}_{it}")
                    nc.vector.tensor_tensor(a[:, 0:w], y, y, AluOpType.mult)
                    b = npool.tile([128, NCHUNK * HPC], F32, tag="nrb",
                                   name=f"nrb{bid}_{it}")
                    nc.vector.tensor_tensor(b[:, 0:w], a[:, 0:w], x,
                                            AluOpType.mult)
                    c = npool.tile([128, NCHUNK * HPC], F32, tag="nrc",
                                   name=f"nrc{bid}_{it}")
                    nc.vector.tensor_scalar(c[:, 0:w], b[:, 0:w], -0.5, 1.5,
                                            AluOpType.mult, AluOpType.add)
                    dst = (npool.tile([128, NCHUNK * HPC], F32, tag="nry",
                                      name=f"nry{bid}_{it}")
                           if it == 0 else None)
                    out = dst[:, 0:w] if it == 0 else rinv_all[:, sl]
                    nc.vector.tensor_tensor(out, y, c[:, 0:w], AluOpType.mult)
                    y = out

            def pass2_pair(kc0, kc1, kvside, act_copies=False):
                # normalize two chunks and transpose them with ONE PE
                # transpose: pre2 cols [h0@kc0 | h1@kc0 | h0@kc1 | h1@kc1]
                dstT = kT if kvside else qT
                pre2 = npool.tile([128, 128], BF, tag="pre2",
                                  name=f"pre2{'kv' if kvside else 'q'}{kc0}")
                for j, kc in enumerate((kc0, kc1)):
                    ci = kc if kvside else KCH + kc
                    kcp = kcp_all[:, ci * HPC * D:(ci + 1) * HPC * D]
                    for h in range(HPC):
                        nc.vector.tensor_scalar(
                            pre2[:, j * 64 + h * D:j * 64 + (h + 1) * D],
                            kcp[:, h * D:(h + 1) * D],
                            rinv_all[:, ci * HPC + h:ci * HPC + h + 1], None,
                            AluOpType.mult)
                    if use_g2 and not kvside:  # q side carries the gq*gk factor
                        nc.vector.tensor_tensor(
                            pre2[:, j * 64:(j + 1) * 64],
                            pre2[:, j * 64:(j + 1) * 64], g2_sb[:, :],
                            AluOpType.mult)
                tp_t = psC.tile([128, 128], BF, tag="c",
                                name=f"tp{'kv' if kvside else 'q'}{kc0}")
                nc.tensor.transpose(tp_t[:, :], pre2[:, :], ident[:, :])
                for j, kc in enumerate((kc0, kc1)):
                    for h in range(HPC):
                        if act_copies:
                            nc.scalar.copy(
                                dstT[h][0:D, kc * 128:(kc + 1) * 128],
                                tp_t[j * 64 + h * D:j * 64 + (h + 1) * D, :])
                        else:
                            nc.vector.tensor_copy(
                                dstT[h][0:D, kc * 128:(kc + 1) * 128],
                                tp_t[j * 64 + h * D:j * 64 + (h + 1) * D, :])

            def norm_group(chunks, kvside):
                # emit a full chain inline (prologue use)
                for kc in chunks:
                    if kvside:
                        pass1(kc, kc, s2T, wkv_sb, wkv_sb2, vx)
                    else:
                        pass1(KCH + kc, kc, s1T, wq_sb, wq_sb2, None)
                ci0 = (chunks[0] if kvside else KCH + chunks[0]) * HPC
                ci1 = (chunks[-1] if kvside else KCH + chunks[-1]) * HPC + HPC
                rsqrt_batch(slice(ci0, ci1), f"{'kv' if kvside else 'q'}{ci0}")
                for i in range(0, len(chunks), 2):
                    pass2_pair(chunks[i], chunks[i + 1], kvside)

            def group_pieces(chunks, kvside, deadline=None, act_copies=False):
                # same chain as norm_group, chopped into filler pieces
                ps = []
                for kc in chunks:
                    if kvside:
                        ps.append((lambda kc=kc: pass1(kc, kc, s2T, wkv_sb, wkv_sb2, vx,
                                                       act_copies=act_copies),
                                   deadline))
                    else:
                        ps.append((lambda kc=kc: pass1(KCH + kc, kc, s1T, wq_sb, wq_sb2, None,
                                                       act_copies=act_copies),
                                   deadline))
                ci0 = (chunks[0] if kvside else KCH + chunks[0]) * HPC
                ci1 = (chunks[-1] if kvside else KCH + chunks[-1]) * HPC + HPC
                ps.append((lambda: rsqrt_batch(slice(ci0, ci1),
                                               f"{'kv' if kvside else 'q'}{ci0}"),
                           deadline))
                for i in range(0, len(chunks), 2):
                    ps.append((lambda i=i: pass2_pair(chunks[i], chunks[i + 1], kvside,
                                                      act_copies=act_copies),
                               deadline))
                return ps

            def proj_out(qc, dge=None):
                # out rows qc*128..: both heads' projection, normalized by 1/Z
                osl = slice(qc * 128, (qc + 1) * 128)
                if qc >= (NQB - 1) * 4:
                    rz0 = rzq[qc % 4][:, 0:1]
                    rz1 = rzq[qc % 4][:, 32:33]
                else:
                    rz0 = rzp[0][:, qc:qc + 1]
                    rz1 = rzp[1][:, qc:qc + 1]
                op0 = psC.tile([128, C_S], F32, tag="c", name=f"op0_{qc}")
                nc.tensor.matmul(op0[:, :], oT_sb[0:D, osl], wout_sb[0:D, :],
                                 start=True, stop=True)
                op1 = psC.tile([128, C_S], F32, tag="c", name=f"op1_{qc}")
                nc.tensor.matmul(op1[:, :], oT_sb[D:2 * D, osl],
                                 wout_sb[D:2 * D, :], start=True, stop=True)
                t0 = work.tile([128, C_S], F32, tag="t0", name=f"t0_{qc}")
                nc.vector.tensor_scalar(t0[:, :], op0[:, :], rz0, None,
                                        AluOpType.mult)
                ops = work.tile([128, C_S], F32, tag="osb", name=f"osb_{qc}")
                nc.vector.scalar_tensor_tensor(
                    ops[:, :], op1[:, :], rz1, t0[:, :],
                    AluOpType.mult, AluOpType.add)
                if dge is not None:
                    nparts = 4 if qc == NQB * 4 - 1 else 2
                    step = 128 // nparts
                    for pi in range(nparts):
                        eng = nc.sync if pi % 2 == 0 else nc.scalar
                        r0 = qc * 128 + pi * step
                        eng.dma_start(out_d.ap()[r0:r0 + step, :],
                                      ops[pi * step:(pi + 1) * step, :])
                else:
                    nc.sync.dma_start(out_d.ap()[osl, :], ops[:, :])

            # ---- attention pipeline ----
            def attend(qb, fillers):
                fillers = list(fillers)
                nf = len(fillers)
                done = 0
                qsl = slice(qb * QB, (qb + 1) * QB)
                oTt = psO.tile([128, QB], F32, tag="oT", name=f"oT{qb}")
                oT = [oTt[OT_OFF[h]:OT_OFF[h] + VW, :] for h in range(HPC)]
                scs = {}
                exs = {}

                def qk(kc):
                    sc = psA.tile([128, HPC * QB], F32, tag="sc",
                                  name=f"sc{qb}_{kc}")
                    for h in range(HPC):
                        nc.tensor.matmul(
                            sc[:, h * QB:(h + 1) * QB],
                            kT[h][:, kc * 128:(kc + 1) * 128],
                            qT[h][:, qsl],
                            start=True, stop=True)
                    scs[kc] = sc

                def pv(kc):
                    ex = exs.pop(kc)
                    for h in range(HPC):
                        nc.tensor.matmul(
                            oT[h][:, :],
                            vx[:, (kc * HPC + h) * VW:(kc * HPC + h + 1) * VW],
                            ex[:, h * QB:(h + 1) * QB],
                            start=(kc == 0), stop=(kc == KCH - 1),
                            skip_group_check=True)

                qk(0)
                for kc in range(KCH):
                    if kc + 1 < KCH:
                        qk(kc + 1)
                    sc = scs.pop(kc)
                    ex = expp.tile([128, HPC * QB], BF, tag="ex",
                                   name=f"ex{qb}_{kc}")
                    nc.scalar.activation(ex[:, :], sc[:, :], AF.Exp, scale=SCALE)
                    exs[kc] = ex
                    while done < nf and (
                            (fillers[done][1] is not None and fillers[done][1] <= kc)
                            or done * KCH < (kc + 1) * nf):
                        fillers[done][0]()
                        done += 1
                    if kc >= 1:
                        pv(kc - 1)
                pv(KCH - 1)
                while done < nf:
                    fillers[done][0]()
                    done += 1

                # epilogue: unnormalized o -> sbuf bf16; Z row -> zrow;
                # 1/Z into q-partition-major layout via a DRAM bounce.  The
                # final q-block instead transposes Z on the (by-then idle) PE
                # so the tail doesn't wait on two serialized DMA round trips.
                if qb == NQB - 1:
                    # tail: fan the epilogue copies across DVE/GpSimd/ACT so
                    # they don't serialize on one engine, and skip zrow (the
                    # DRAM bounce isn't used for the final q-block).
                    nc.scalar.copy(oT_sb[0 * D:1 * D, qsl], oT[0][0:D, :])
                    nc.scalar.copy(oT_sb[1 * D:2 * D, qsl], oT[1][0:D, :])
                    nc.vector.tensor_copy(zq_bf[0:1, :], oT[0][D:VW, :])
                    nc.vector.tensor_copy(zq_bf[32:33, :], oT[1][D:VW, :])
                    for qc4 in range(4):
                        tpz = psC.tile([128, 64], BF, tag="c", name=f"tpz{qc4}")
                        nc.tensor.transpose(
                            tpz[:, 0:33],
                            zq_bf[0:33, qc4 * 128:(qc4 + 1) * 128],
                            ident[0:33, 0:33])
                        nc.vector.reciprocal(
                            rzq[qc4][:, :], tpz[:, 0:33])
                else:
                    for h in range(HPC):
                        nc.vector.tensor_copy(oT_sb[h * D:(h + 1) * D, qsl],
                                              oT[h][0:D, :])
                        nc.vector.tensor_copy(zrow[h][0:1, qsl],
                                              oT[h][D:VW, :])
                    for h in range(HPC):
                        nc.sync.dma_start(zdr[h].ap()[0:1, qsl], zrow[h][0:1, qsl])
                        nc.sync.dma_start(
                            zp[h][:, qb * 4:(qb + 1) * 4],
                            zdr[h].ap()[0:1, qsl].rearrange("o (c p) -> o p c", p=128)[0])
                        nc.vector.reciprocal(rzp[h][:, qb * 4:(qb + 1) * 4],
                                             zp[h][:, qb * 4:(qb + 1) * 4])

            # ---- schedule ----
            # prologue: kv chunks 0-3 + q chunks 0-3 fully normalized.
            # All pass1 chains go first (they only gate on staging slices);
            # the rsqrt batches land in the window where the pad-row memsets
            # still hog the engines, so nothing downstream waits on them.
            for kc in range(4):
                pass1(kc, kc, s2T, wkv_sb, wkv_sb2, vx)
                pass1(KCH + kc, kc, s1T, wq_sb, wq_sb2, None)
            rsqrt_batch(slice(0, 8), "kv0")
            rsqrt_batch(slice(KCH * HPC, KCH * HPC + 8), "q0")
            pass2_pair(0, 1, False, act_copies=True)
            pass2_pair(2, 3, False, act_copies=True)
            pass2_pair(0, 1, True, act_copies=True)

            # attend(0): kv chunks 4-23 JIT (chunk c's pieces forced by step
            # c-2 so the QK that reads kT[c] sees the writes), then q 4-7.
            f0 = [(lambda: pass2_pair(2, 3, True, act_copies=True), 0)]
            for g0 in range(4, 24, 4):
                g = list(range(g0, g0 + 4))
                f0 += group_pieces(g, kvside=True, deadline=max(0, g[0] - 2))
            f0 += group_pieces([4, 5, 6, 7], kvside=False)
            attend(0, f0)

            # attend(1..4): one q-chunk group each; attend(2+) also carry the
            # out-projections of qb-2 (their 1/Z landed ~a full attend ago).
            for qb in range(1, NQB):
                fq = []
                g0 = 4 + 4 * qb
                if g0 < QCH:
                    fq += group_pieces(list(range(g0, g0 + 4)), kvside=False)
                if qb >= 2:
                    for qc in range((qb - 2) * 4, (qb - 1) * 4):
                        fq.append((lambda qc=qc: proj_out(qc), None))
                if qb == NQB - 1:
                    for qc in range((qb - 1) * 4, qb * 4):
                        fq.append((lambda qc=qc: proj_out(qc), None))
                attend(qb, fq)
            for qc in range((NQB - 1) * 4, NQB * 4):
                proj_out(qc, dge=True)

    nc.compile()
    return nc


def _host_prep(inputs):
    s1 = np.asarray(inputs["s1"], np.float32)
    s2 = np.asarray(inputs["s2"], np.float32)
    ridx1 = np.asarray(inputs["ridx1"], np.int32)
    ct1 = np.asarray(inputs["ct1"], np.int32)
    mask1 = np.asarray(inputs["mask1"], np.int32)
    mask2 = np.asarray(inputs["mask2"], np.int32)
    Wq = np.asarray(inputs["Wq"], np.float32)
    Wkv = np.asarray(inputs["Wkv"], np.float32)
    Wout = np.asarray(inputs["Wout"], np.float32)
    gq = np.asarray(inputs["gq"], np.float32)
    gk = np.asarray(inputs["gk"], np.float32)

    ct_idx = np.take_along_axis(ridx1, ct1[:, None], axis=1)
    pos = (ridx1 - ct_idx).astype(np.float32)
    half = C_S // 2
    freqs = np.exp(-np.log(10000.0) * np.arange(half, dtype=np.float32) / half)
    ang = pos[..., None] * freqs
    s1e = s1 + np.concatenate([np.sin(ang), np.cos(ang)], axis=-1).astype(np.float32)

    m1 = mask1.astype(np.float32)
    km = (mask2.astype(np.float32) - 1.0) * INF / SCALE

    g2 = gq * gk
    use_g2 = not np.allclose(g2, 1.0)

    ident = np.eye(128, dtype=bf16)
    in_maps = []
    for c in range(NCORES):
        b, hp = c // 4, c % 4
        m = {
            "s1T": np.ascontiguousarray(s1e[b].T).astype(bf16),
            "s2T": np.ascontiguousarray(s2[b].T).astype(bf16),
            "wq": np.ascontiguousarray(Wq[:, hp * HPC * D:(hp + 1) * HPC * D]).astype(bf16),
            "wkv": np.ascontiguousarray(Wkv[:, hp * HPC * 2 * D:(hp + 1) * HPC * 2 * D]).astype(bf16),
            "wout": np.ascontiguousarray(Wout[hp * HPC * D:(hp + 1) * HPC * D, :]).astype(bf16),
            "qm": m1[b][None, :].astype(bf16),
            "km": km[b][None, :].astype(bf16),
            "ident": ident,
        }
        if use_g2:
            m["g2"] = np.tile(g2[None, hp * HPC * D:(hp + 1) * HPC * D], (128, 1)).astype(bf16)
        in_maps.append(m)
    return in_maps, use_g2, np.asarray(inputs["b_out"], np.float32)


def _run(inputs, trace=False, **kw):
    in_maps, use_g2, b_out = _host_prep(inputs)
    key = ("nc", use_g2)
    if key not in _cache:
        _cache[key] = _build(use_g2)
    nc = _cache[key]
    res = bass_utils.run_bass_kernel_spmd(
        nc, in_maps, core_ids=list(range(NCORES)), trace=trace, **kw)
    out = np.zeros((B, N1, C_S), np.float32)
    for c in range(NCORES):
        out[c // 4] += res.results[c]["out"]
    out += b_out[None, None, :]
    return out, res


def kernel(**inputs) -> np.ndarray:
    out, _ = _run(inputs, trace=False)
    return out
